# Initial kernel scaffold
#
"""Causal self-attention Trainium2 Bass kernel.

Problem: x[4, 2048, 1024], 16 heads, head_dim 64; y = softmax_causal(
(xWq.T)(xWk.T)^T / sqrt(C)) (xWv.T) Wo.T + bo.

Sharding over 8 NeuronCores: core = (batch b, head-group g) with 4 batches x
2 groups of 8 heads. Each core computes its group's Q/K/V projections,
causal attention, and a partial output projection (its 512 columns of the
feature dim); host sums the two partials per batch and adds the bias.

Per-core layouts (all "transposed", feature-on-partition):
  xT  [1024, 2048]  = x[b].T
  wqT/wkT/wvT [1024, 512] = Wq[g-rows].T      (y = x @ W.T)
  woT [512, 1024] = Wo[:, g-cols].T
  pT  [1024, 2048]  output partial, transposed

Matmuls run as float32r (TF32-like, 1 cycle/row at N>=512 vs 4 for fp32).
Softmax skips max-subtraction (logits are O(1) by construction: Wq,Wk scale
1/sqrt(C)); denominator comes free as a ones-column appended to V.
"""

import numpy as np
import concourse.bacc as bacc
import concourse.tile as tile
from concourse import mybir
from concourse.bass_utils import run_bass_kernel_spmd

N, T, C, H, D = 4, 2048, 1024, 16, 64
G = 2                 # head groups (tensor-parallel factor)
HG = H // G           # 8 heads per group
F = HG * D            # 512 features per group
NCORES = N * G        # 8
CHUNK = 512           # token chunk (q-chunk == projection t-chunk)
NCH = T // CHUNK      # 4
CT = C // 128         # 8 contraction tiles for projections
MT = F // 128         # 4 feature m-tiles per group
JT = C // 128         # 8 output-channel j-tiles

F32 = mybir.dt.float32
F32R = mybir.dt.float32r
EXP = mybir.ActivationFunctionType.Exp

_NC_CACHE = {}


def _emit(nc, tc, ctx, xT, wqT, wkT, wvT, woT, cmask, pT, r):
    """Emit one full forward pass. r = repeat index (names must be unique)."""
    import contextlib

    persist = ctx.enter_context(tc.tile_pool(name=f"persist{r}", bufs=1))
    wpool = ctx.enter_context(tc.tile_pool(name=f"w{r}", bufs=2))
    xtpool = ctx.enter_context(tc.tile_pool(name=f"xt{r}", bufs=1))
    qtpool = ctx.enter_context(tc.tile_pool(name=f"qt{r}", bufs=2))
    expool = ctx.enter_context(tc.tile_pool(name=f"ex{r}", bufs=2))
    otpool = ctx.enter_context(tc.tile_pool(name=f"ot{r}", bufs=2))
    bcpool = ctx.enter_context(tc.tile_pool(name=f"bc{r}", bufs=2))
    rcpool = ctx.enter_context(tc.tile_pool(name=f"rc{r}", bufs=2))
    ps_proj = ctx.enter_context(tc.tile_pool(name=f"psp{r}", bufs=2, space="PSUM"))
    ps_st = ctx.enter_context(tc.tile_pool(name=f"psst{r}", bufs=2, space="PSUM"))
    ps_o = ctx.enter_context(tc.tile_pool(name=f"pso{r}", bufs=2, space="PSUM"))

    # ---- persistent tiles ----
    # K^T per (chunk, m-tile): [128 feat, 512 tok]
    kt_sb = [[persist.tile([128, CHUNK], F32R, name=f"kt{r}_{ch}_{m}",
                           tag=f"kt{r}_{ch}_{m}")
              for m in range(MT)] for ch in range(NCH)]
    # V natural layout per token-tile: 8 heads x (64 cols + ones col)
    v_sb = [persist.tile([128, HG * (D + 1)], F32R, name=f"v{r}_{tt}",
                         tag=f"v{r}_{tt}") for tt in range(T // 128)]
    # Wo^T k-tiles: [128 feat, 1024 out-ch]
    wo_sb = [persist.tile([128, C], F32R, name=f"wo{r}_{k}", tag=f"wo{r}_{k}")
             for k in range(MT)]
    # causal 0/1 masks for the 4 diagonal positions
    cm_sb = [persist.tile([128, CHUNK], F32R, name=f"cm{r}_{p}", tag=f"cm{r}_{p}")
             for p in range(4)]

    for k in range(MT):
        nc.sync.dma_start(out=wo_sb[k][:], in_=woT[128 * k:128 * (k + 1), :])
    for p in range(4):
        nc.sync.dma_start(out=cm_sb[p][:], in_=cmask[p])
    for tt in range(T // 128):
        nc.vector.memset(v_sb[tt][:].bitcast(F32), 1.0)

    for ch in range(NCH):
        tsl = slice(CHUNK * ch, CHUNK * (ch + 1))
        # ---------------- projections for this chunk ----------------
        xt_sb = [xtpool.tile([128, CHUNK], F32R, name=f"x{r}_{ch}_{c}",
                             tag=f"x{c}") for c in range(CT)]
        for c in range(CT):
            nc.sync.dma_start(out=xt_sb[c][:], in_=xT[128 * c:128 * (c + 1), tsl])

        qt_t = []
        for mat, wsrc in (("q", wqT), ("k", wkT)):
            w_sb = [wpool.tile([128, F], F32R, name=f"w{mat}{r}_{ch}_{c}",
                               tag=f"w{c}") for c in range(CT)]
            for c in range(CT):
                nc.sync.dma_start(out=w_sb[c][:], in_=wsrc[128 * c:128 * (c + 1), :])
            for m in range(MT):
                psq = ps_proj.tile([128, CHUNK], F32, name=f"ps{mat}{r}_{ch}_{m}",
                                   tag="proj")
                for c in range(CT):
                    nc.tensor.matmul(psq[:], w_sb[c][:, 128 * m:128 * (m + 1)],
                                     xt_sb[c][:], start=(c == 0), stop=(c == CT - 1))
                if mat == "q":
                    dst = qtpool.tile([128, CHUNK], F32R, name=f"qt{r}_{ch}_{m}",
                                      tag=f"qt{m}")
                    qt_t.append(dst)
                    nc.vector.tensor_copy(dst[:], psq[:])
                else:
                    nc.vector.tensor_copy(kt_sb[ch][m][:], psq[:])

        wv_sb = [wpool.tile([128, F], F32R, name=f"wv{r}_{ch}_{c}", tag=f"w{c}")
                 for c in range(CT)]
        for c in range(CT):
            nc.sync.dma_start(out=wv_sb[c][:], in_=wvT[128 * c:128 * (c + 1), :])
        for t4 in range(4):
            tt = 4 * ch + t4
            psv = ps_proj.tile([128, F], F32, name=f"psv{r}_{tt}", tag="proj")
            for c in range(CT):
                nc.tensor.matmul(psv[:], xt_sb[c][:, 128 * t4:128 * (t4 + 1)],
                                 wv_sb[c][:], start=(c == 0), stop=(c == CT - 1))
            # strided copy into [128, 8, 65][:, :, 0:64], keeping ones columns
            dst = v_sb[tt][:].rearrange("p (h e) -> p h e", e=D + 1)[:, :, 0:D]
            src = psv[:].rearrange("p (h e) -> p h e", e=D)
            nc.vector.tensor_copy(dst, src)

        # ---------------- attention for q-chunk == ch ----------------
        nkt = 4 * ch + 4          # k-tiles needed (even count)
        ot_t = []
        for h in range(HG):
            mp, row0 = h // 2, 64 * (h % 2)
            qt_h = qt_t[mp][row0:row0 + 64, :]
            o_ps = ps_o.tile([65, CHUNK], F32, name=f"o{r}_{ch}_{h}", tag="o")
            for kp in range(0, nkt, 2):
                stp = ps_st.tile([128, 2 * CHUNK], F32,
                                 name=f"st{r}_{ch}_{h}_{kp}", tag="st")
                ex = expool.tile([128, 2 * CHUNK], F32R,
                                 name=f"ex{r}_{ch}_{h}_{kp}", tag="ex")
                for half in range(2):
                    kt = kp + half
                    ktch, ktin = kt // 4, kt % 4
                    lhs = kt_sb[ktch][mp][row0:row0 + 64,
                                          128 * ktin:128 * (ktin + 1)]
                    nc.tensor.matmul(stp[:, CHUNK * half:CHUNK * (half + 1)],
                                     lhs, qt_h, start=True, stop=True)
                nc.scalar.activation(out=ex[:], in_=stp[:], func=EXP,
                                     scale=1.0 / 32.0)
                for half in range(2):
                    kt = kp + half
                    p = kt - 4 * ch
                    exh = ex[:, CHUNK * half:CHUNK * (half + 1)]
                    if p >= 0:
                        nc.vector.tensor_mul(exh, exh, cm_sb[p][:])
                    nc.tensor.matmul(o_ps[:],
                                     v_sb[kt][:, (D + 1) * h:(D + 1) * (h + 1)],
                                     exh, start=(kt == 0), stop=(kt == nkt - 1))
            # normalize: rows 0..63 / row 64
            rc = rcpool.tile([1, CHUNK], F32, name=f"rc{r}_{ch}_{h}", tag="rc")
            nc.vector.reciprocal(rc[:], o_ps[64:65, :])
            bc = bcpool.tile([128, CHUNK], F32, name=f"bc{r}_{ch}_{h}", tag="bc")
            nc.gpsimd.partition_broadcast(bc[:], rc[:])
            if h % 2 == 0:
                ot = otpool.tile([128, CHUNK], F32R, name=f"otn{r}_{ch}_{mp}",
                                 tag=f"ot{mp}")
                ot_t.append(ot)
            dst = ot_t[mp][row0:row0 + 64, :]
            nc.vector.tensor_copy(dst, o_ps[0:64, :])
            nc.vector.tensor_mul(dst, dst, bc[row0:row0 + 64, :])

        # ---------------- output projection for this chunk ----------------
        for j in range(JT):
            psp = ps_proj.tile([128, CHUNK], F32, name=f"pso{r}_{ch}_{j}",
                               tag="proj")
            for k in range(MT):
                nc.tensor.matmul(psp[:], wo_sb[k][:, 128 * j:128 * (j + 1)],
                                 ot_t[k][:], start=(k == 0), stop=(k == MT - 1))
            nc.sync.dma_start(out=pT[128 * j:128 * (j + 1), tsl], in_=psp[:])


def _build(repeat=1):
    from contextlib import ExitStack

    nc = bacc.Bacc("TRN2", target_bir_lowering=False, debug=False)
    xT = nc.dram_tensor("xT", [C, T], F32R, kind="ExternalInput")
    wqT = nc.dram_tensor("wqT", [C, F], F32R, kind="ExternalInput")
    wkT = nc.dram_tensor("wkT", [C, F], F32R, kind="ExternalInput")
    wvT = nc.dram_tensor("wvT", [C, F], F32R, kind="ExternalInput")
    woT = nc.dram_tensor("woT", [F, C], F32R, kind="ExternalInput")
    cmask = nc.dram_tensor("cmask", [4, 128, CHUNK], F32R, kind="ExternalInput")
    pT = nc.dram_tensor("pT", [C, T], F32, kind="ExternalOutput")

    with tile.TileContext(nc) as tc:
        for r in range(repeat):
            with ExitStack() as ctx:
                _emit(nc, tc, ctx, xT, wqT, wkT, wvT, woT, cmask, pT, r)
    nc.compile()
    return nc


def _get_nc(repeat=1):
    if repeat not in _NC_CACHE:
        _NC_CACHE[repeat] = _build(repeat)
    return _NC_CACHE[repeat]


def _make_cmask():
    p = np.arange(4)[:, None, None]
    k = np.arange(128)[None, :, None]
    q = np.arange(CHUNK)[None, None, :]
    return (128 * p + k <= q).astype(np.float32)


def _in_maps(x, Wq, Wk, Wv, Wo):
    cmask = _make_cmask()
    maps = []
    for b in range(N):
        xT = np.ascontiguousarray(x[b].T)
        for g in range(G):
            sl = slice(g * F, (g + 1) * F)
            maps.append({
                "xT": xT,
                "wqT": np.ascontiguousarray(Wq[sl].T),
                "wkT": np.ascontiguousarray(Wk[sl].T),
                "wvT": np.ascontiguousarray(Wv[sl].T),
                "woT": np.ascontiguousarray(Wo[:, sl].T),
                "cmask": cmask,
            })
    return maps


def kernel(x, Wq, Wk, Wv, Wo, bo, _repeat=1):
    x = np.asarray(x, dtype=np.float32)
    Wq = np.asarray(Wq, dtype=np.float32)
    Wk = np.asarray(Wk, dtype=np.float32)
    Wv = np.asarray(Wv, dtype=np.float32)
    Wo = np.asarray(Wo, dtype=np.float32)
    bo = np.asarray(bo, dtype=np.float32)

    nc = _get_nc(_repeat)
    res = run_bass_kernel_spmd(nc, _in_maps(x, Wq, Wk, Wv, Wo),
                               list(range(NCORES)))
    out = np.empty((N, T, C), dtype=np.float32)
    for b in range(N):
        acc = res.results[G * b]["pT"].astype(np.float32)
        for g in range(1, G):
            acc = acc + res.results[G * b + g]["pT"]
        out[b] = acc.T + bo
    return out


# revision 3
# speedup vs baseline: 5.7101x; 5.7101x over previous
"""Causal self-attention Trainium2 Bass kernel.

Problem: x[4, 2048, 1024], 16 heads, head_dim 64; y = softmax_causal(
(xWq.T)(xWk.T)^T / sqrt(C)) (xWv.T) Wo.T + bo.

Sharding over 8 NeuronCores: core = (batch b, head-group g) with 4 batches x
2 groups of 8 heads. Each core computes its group's Q/K/V projections,
causal attention, and a partial output projection (its 512 columns of the
feature dim); host sums the two partials per batch and adds the bias.

Per-core layouts (all "transposed", feature-on-partition):
  xT  [1024, 2048]  = x[b].T
  wqT/wkT/wvT [1024, 512] = Wq[g-rows].T      (y = x @ W.T)
  woT [512, 1024] = Wo[:, g-cols].T
  pT  [1024, 2048]  output partial, transposed

Matmuls run as float32r (TF32-like, 1 cycle/row at N>=512 vs 4 for fp32).
Softmax skips max-subtraction (logits are O(1) by construction: Wq,Wk scale
1/sqrt(C)); denominator comes free as a ones-column appended to V.
"""

import numpy as np
import concourse.bacc as bacc
import concourse.tile as tile
from concourse import mybir
from concourse.bass_utils import run_bass_kernel_spmd

N, T, C, H, D = 4, 2048, 1024, 16, 64
G = 2                 # head groups (tensor-parallel factor)
HG = H // G           # 8 heads per group
F = HG * D            # 512 features per group
NCORES = N * G        # 8
CHUNK = 512           # token chunk (q-chunk == projection t-chunk)
NCH = T // CHUNK      # 4
CT = C // 128         # 8 contraction tiles for projections
MT = F // 128         # 4 feature m-tiles per group
JT = C // 128         # 8 output-channel j-tiles

F32 = mybir.dt.float32
F32R = mybir.dt.float32r
EXP = mybir.ActivationFunctionType.Exp

_NC_CACHE = {}


def _emit(nc, tc, ctx, xT, wqT, wkT, wvT, woT, cmask, pT, r):
    """Emit one full forward pass. r = repeat index (names must be unique)."""
    import contextlib

    persist = ctx.enter_context(tc.tile_pool(name=f"persist{r}", bufs=1))
    wpool = ctx.enter_context(tc.tile_pool(name=f"w{r}", bufs=2))
    xtpool = ctx.enter_context(tc.tile_pool(name=f"xt{r}", bufs=1))
    qtpool = ctx.enter_context(tc.tile_pool(name=f"qt{r}", bufs=2))
    expool = ctx.enter_context(tc.tile_pool(name=f"ex{r}", bufs=2))
    otpool = ctx.enter_context(tc.tile_pool(name=f"ot{r}", bufs=2))
    bcpool = ctx.enter_context(tc.tile_pool(name=f"bc{r}", bufs=2))
    rcpool = ctx.enter_context(tc.tile_pool(name=f"rc{r}", bufs=2))
    oppool = ctx.enter_context(tc.tile_pool(name=f"op{r}", bufs=2))
    ps_proj = ctx.enter_context(tc.tile_pool(name=f"psp{r}", bufs=2, space="PSUM"))
    ps_st = ctx.enter_context(tc.tile_pool(name=f"psst{r}", bufs=2, space="PSUM"))
    ps_o = ctx.enter_context(tc.tile_pool(name=f"pso{r}", bufs=2, space="PSUM"))

    # ---- persistent tiles ----
    # K^T per (chunk, m-tile): [128 feat, 512 tok]
    kt_sb = [[persist.tile([128, CHUNK], F32R, name=f"kt{r}_{ch}_{m}",
                           tag=f"kt{r}_{ch}_{m}")
              for m in range(MT)] for ch in range(NCH)]
    # V natural layout per token-tile: 8 heads x (64 cols + ones col)
    v_sb = [persist.tile([128, HG * (D + 1)], F32R, name=f"v{r}_{tt}",
                         tag=f"v{r}_{tt}") for tt in range(T // 128)]
    # Wo^T k-tiles: [128 feat, 1024 out-ch]
    wo_sb = [persist.tile([128, C], F32R, name=f"wo{r}_{k}", tag=f"wo{r}_{k}")
             for k in range(MT)]
    # causal 0/1 masks for the 4 diagonal positions
    cm_sb = [persist.tile([128, CHUNK], F32R, name=f"cm{r}_{p}", tag=f"cm{r}_{p}")
             for p in range(4)]

    for k in range(MT):
        nc.sync.dma_start(out=wo_sb[k][:], in_=woT[128 * k:128 * (k + 1), :])
    for p in range(4):
        nc.sync.dma_start(out=cm_sb[p][:], in_=cmask[p])
    for tt in range(T // 128):
        nc.vector.memset(v_sb[tt][:].bitcast(F32), 1.0)

    for ch in range(int(__import__('os').environ.get('KDBG_NCH', NCH))):
        tsl = slice(CHUNK * ch, CHUNK * (ch + 1))
        # ---------------- projections for this chunk ----------------
        xt_sb = [xtpool.tile([128, CHUNK], F32R, name=f"x{r}_{ch}_{c}",
                             tag=f"x{c}") for c in range(CT)]
        for c in range(CT):
            nc.sync.dma_start(out=xt_sb[c][:], in_=xT[128 * c:128 * (c + 1), tsl])

        qt_t = []
        for mat, wsrc in (("q", wqT), ("k", wkT)):
            w_sb = [wpool.tile([128, F], F32R, name=f"w{mat}{r}_{ch}_{c}",
                               tag=f"w{c}") for c in range(CT)]
            for c in range(CT):
                nc.sync.dma_start(out=w_sb[c][:], in_=wsrc[128 * c:128 * (c + 1), :])
            for m in range(MT):
                psq = ps_proj.tile([128, CHUNK], F32, name=f"ps{mat}{r}_{ch}_{m}",
                                   tag="proj")
                for c in range(CT):
                    nc.tensor.matmul(psq[:], w_sb[c][:, 128 * m:128 * (m + 1)],
                                     xt_sb[c][:], start=(c == 0), stop=(c == CT - 1))
                if mat == "q":
                    dst = qtpool.tile([128, CHUNK], F32R, name=f"qt{r}_{ch}_{m}",
                                      tag=f"qt{m}")
                    qt_t.append(dst)
                    nc.vector.tensor_copy(dst[:], psq[:])
                else:
                    nc.vector.tensor_copy(kt_sb[ch][m][:], psq[:])

        wv_sb = [wpool.tile([128, F], F32R, name=f"wv{r}_{ch}_{c}", tag=f"w{c}")
                 for c in range(CT)]
        for c in range(CT):
            nc.sync.dma_start(out=wv_sb[c][:], in_=wvT[128 * c:128 * (c + 1), :])
        for t4 in range(4):
            tt = 4 * ch + t4
            psv = ps_proj.tile([128, F], F32, name=f"psv{r}_{tt}", tag="proj")
            for c in range(CT):
                nc.tensor.matmul(psv[:], xt_sb[c][:, 128 * t4:128 * (t4 + 1)],
                                 wv_sb[c][:], start=(c == 0), stop=(c == CT - 1))
            # strided copy into [128, 8, 65][:, :, 0:64], keeping ones columns
            dst = v_sb[tt][:].rearrange("p (h e) -> p h e", e=D + 1)[:, :, 0:D]
            src = psv[:].rearrange("p (h e) -> p h e", e=D)
            nc.vector.tensor_copy(dst, src)

        # ---------------- attention for q-chunk == ch ----------------
        nkt = 4 * ch + 4          # k-tiles needed (even count)
        ot_t = []
        for h in range(HG):
            mp, row0 = h // 2, 64 * (h % 2)
            qt_h = qt_t[mp][row0:row0 + 64, :]
            o_ps = ps_o.tile([65, CHUNK], F32, name=f"o{r}_{ch}_{h}", tag="o")
            for kp in range(0, nkt, 2):
                stp = ps_st.tile([128, 2 * CHUNK], F32,
                                 name=f"st{r}_{ch}_{h}_{kp}", tag="st")
                ex = expool.tile([128, 2 * CHUNK], F32R,
                                 name=f"ex{r}_{ch}_{h}_{kp}", tag="ex")
                for half in range(2):
                    kt = kp + half
                    ktch, ktin = kt // 4, kt % 4
                    lhs = kt_sb[ktch][mp][row0:row0 + 64,
                                          128 * ktin:128 * (ktin + 1)]
                    nc.tensor.matmul(stp[:, CHUNK * half:CHUNK * (half + 1)],
                                     lhs, qt_h, start=True, stop=True)
                nc.scalar.activation(out=ex[:], in_=stp[:], func=EXP,
                                     scale=1.0 / 32.0)
                for half in range(2):
                    kt = kp + half
                    p = kt - 4 * ch
                    exh = ex[:, CHUNK * half:CHUNK * (half + 1)]
                    if p >= 0:
                        nc.vector.tensor_mul(exh, exh, cm_sb[p][:])
                    nc.tensor.matmul(o_ps[:],
                                     v_sb[kt][:, (D + 1) * h:(D + 1) * (h + 1)],
                                     exh, start=(kt == 0), stop=(kt == nkt - 1))
            # normalize: rows 0..63 / row 64
            rc = rcpool.tile([1, CHUNK], F32, name=f"rc{r}_{ch}_{h}", tag="rc")
            nc.vector.reciprocal(rc[:], o_ps[64:65, :])
            bc = bcpool.tile([128, CHUNK], F32, name=f"bc{r}_{ch}_{h}", tag="bc")
            nc.gpsimd.partition_broadcast(bc[:], rc[:])
            if h % 2 == 0:
                ot = otpool.tile([128, CHUNK], F32R, name=f"otn{r}_{ch}_{mp}",
                                 tag=f"ot{mp}")
                ot_t.append(ot)
            dst = ot_t[mp][row0:row0 + 64, :]
            nc.vector.tensor_copy(dst, o_ps[0:64, :])
            nc.vector.tensor_mul(dst, dst, bc[row0:row0 + 64, :])

        # ---------------- output projection for this chunk ----------------
        for j in range(JT):
            psp = ps_proj.tile([128, CHUNK], F32, name=f"pso{r}_{ch}_{j}",
                               tag="proj")
            for k in range(MT):
                nc.tensor.matmul(psp[:], wo_sb[k][:, 128 * j:128 * (j + 1)],
                                 ot_t[k][:], start=(k == 0), stop=(k == MT - 1))
            ob = oppool.tile([128, CHUNK], F32, name=f"ob{r}_{ch}_{j}", tag="ob")
            nc.vector.tensor_copy(ob[:], psp[:])
            nc.sync.dma_start(out=pT[128 * j:128 * (j + 1), tsl], in_=ob[:])


def _build(repeat=1):
    from contextlib import ExitStack

    nc = bacc.Bacc("TRN2", target_bir_lowering=False, debug=False)
    xT = nc.dram_tensor("xT", [C, T], F32R, kind="ExternalInput")
    wqT = nc.dram_tensor("wqT", [C, F], F32R, kind="ExternalInput")
    wkT = nc.dram_tensor("wkT", [C, F], F32R, kind="ExternalInput")
    wvT = nc.dram_tensor("wvT", [C, F], F32R, kind="ExternalInput")
    woT = nc.dram_tensor("woT", [F, C], F32R, kind="ExternalInput")
    cmask = nc.dram_tensor("cmask", [4, 128, CHUNK], F32R, kind="ExternalInput")
    pT = nc.dram_tensor("pT", [C, T], F32, kind="ExternalOutput")

    with tile.TileContext(nc) as tc:
        for r in range(repeat):
            with ExitStack() as ctx:
                _emit(nc, tc, ctx, xT, wqT, wkT, wvT, woT, cmask, pT, r)
    nc.compile()
    return nc


def _get_nc(repeat=1):
    if repeat not in _NC_CACHE:
        _NC_CACHE[repeat] = _build(repeat)
    return _NC_CACHE[repeat]


def _make_cmask():
    p = np.arange(4)[:, None, None]
    k = np.arange(128)[None, :, None]
    q = np.arange(CHUNK)[None, None, :]
    return (128 * p + k <= q).astype(np.float32)


def _in_maps(x, Wq, Wk, Wv, Wo):
    cmask = _make_cmask()
    maps = []
    for b in range(N):
        xT = np.ascontiguousarray(x[b].T)
        for g in range(G):
            sl = slice(g * F, (g + 1) * F)
            maps.append({
                "xT": xT,
                "wqT": np.ascontiguousarray(Wq[sl].T),
                "wkT": np.ascontiguousarray(Wk[sl].T),
                "wvT": np.ascontiguousarray(Wv[sl].T),
                "woT": np.ascontiguousarray(Wo[:, sl].T),
                "cmask": cmask,
            })
    return maps


def kernel(x, Wq, Wk, Wv, Wo, bo, _repeat=1):
    x = np.asarray(x, dtype=np.float32)
    Wq = np.asarray(Wq, dtype=np.float32)
    Wk = np.asarray(Wk, dtype=np.float32)
    Wv = np.asarray(Wv, dtype=np.float32)
    Wo = np.asarray(Wo, dtype=np.float32)
    bo = np.asarray(bo, dtype=np.float32)

    nc = _get_nc(_repeat)
    res = run_bass_kernel_spmd(nc, _in_maps(x, Wq, Wk, Wv, Wo),
                               list(range(NCORES)))
    out = np.empty((N, T, C), dtype=np.float32)
    for b in range(N):
        acc = res.results[G * b]["pT"].astype(np.float32)
        for g in range(1, G):
            acc = acc + res.results[G * b + g]["pT"]
        out[b] = acc.T + bo
    return out
